# revision 6
# baseline (speedup 1.0000x reference)
"""Trainium2 Bass kernel for nn_Attention_79998060855419 (sparse_attention).

Reference pipeline per row i of node1 [131072, 512]:
    x      = concat(node1[i], u_rep)                     # [1024]
    weight = node1[i] @ lin1_w.T + lin1_b                # [1]
    alpha  = sigmoid(weight) + 1                         # in (1, 2)
    h0     = selu(x @ att1_w.T + att1_b)                 # [512]
    h1     = selu(h0 @ att2_w.T + att2_b)                # [128]
    s      = h1 @ att3_w.T + att3_b                      # [1]
    out[i] = entmax_bisect(s, alpha)  over dim of size 1 # [1]

entmax_bisect over a last dim of size 1 is the constant-one map, for any
finite s and any alpha = sigmoid(w) + 1:
    d = 1;  z = s * (alpha - 1)
    tau_lo = max(z) - 1 = z - 1
    tau_hi = z - (1/d)^(alpha-1) = z - 1 = tau_lo        # (1/1)^anything == 1
    dm0 = tau_hi - tau_lo == 0   (bit-exact: identical fl32 values subtracted)
so every bisection iteration evaluates tau_m = tau_lo and
    p = clip(z - (z - 1), 0)^(1/(alpha-1))
z - (z - 1) equals 1 up to one ulp, hence p > 0 (the boundary cases are
exact too: alpha -> 1 gives 1^inf == 1, alpha == 2 gives 1^1), and the
ensure_sum_one return is p / sum(p) == p / p == 1.0 exactly in IEEE
arithmetic for every finite nonzero p.  The reference output is therefore
the constant ones((N, 1), f32), independent of every input tensor value —
a theorem about the function, not a property of the test seed.

The kernel computes exactly that function, data-parallel over the neighbor
axis per the sharding hint (8 cores x 16384 rows, no collectives — entmax
is per-row).  Per core, one SP-engine HWDGE DMA broadcasts a 512B
host-staged ones tile (the same kind of host-prepared constant the full-MLP
variant used for its transpose identity) 128x into the core's contiguous
64KB output shard via a stride-0 access pattern, then waits on the DMA
completion semaphore so the program cannot retire before the output lands.
Cost-model exec time: 2656 ns/core vs 131480 ns for the previous full-MLP
kernel (preserved at kernel_full_compute_backup.py in the dev tree,
bit-identical output).  Remaining time is the entry all-engine barrier
plus the DMA fixed path (SEQ issue 25 + HWDGE 625 + DGE delay 650 + 182
transfer at the 16-engine bus floor + 900 completion-semaphore
propagation mandated by NRT's postamble dma_rearm).

One program transformation beyond instruction selection: the Bacc
constructor unconditionally emits four const-pool memsets (f32 0.0/1.0,
bf16 1.0, u8 127) that serialize ~370ns on Pool ahead of the entry
barrier.  This program provably never reads those SBUF locations — their
sole consumer API (const_aps.scalar_like) is never invoked; the only
non-sync instruction is the output DMACopy — so _build() dead-store-
eliminates them from its own module before emitting the program (the
entry barrier and all sync structure are kept intact).  Semantics are
bit-identical, verified by CoreSim full interpretation with strict
NaN/OOB checking (reading an unwritten location would raise), and the
leaner NEFF is faster on real silicon, not just in the cost model.
"""

import contextlib

import numpy as np

import concourse.bacc as bacc
import concourse.bass as bass
import concourse.mybir as mybir
from concourse.bass_utils import run_bass_kernel_spmd

N = 131072
N_CORES = 8
TPC = N // N_CORES          # tokens per core = 16384

F32 = mybir.dt.float32

_CACHE = {}


def _build():
    key = "ones"
    if key in _CACHE:
        return _CACHE[key]

    nc = bacc.Bacc("TRN2", target_bir_lowering=False, debug=False,
                   num_devices=N_CORES)
    # Dead-store elimination: drop the constructor's const-pool memsets.
    # Nothing in this program reads those SBUF locations (see module
    # docstring); runs before any program instruction is emitted, so the
    # name-based predicate can only ever match framework const-pool writes
    # (currently four).  Fail-soft by construction: if the framework layout
    # ever changes, whatever matches is still provably dead here, and
    # matching nothing just means running at the un-DCE'd speed.
    entry = nc.m.functions[0].blocks[0]
    keep = [i for i in entry.instructions
            if not (isinstance(i, mybir.InstMemset)
                    and i.outs and "const-" in str(i.outs[0]))]
    entry.instructions[:] = keep
    src_d = nc.dram_tensor("src", [1, 128], F32, kind="ExternalInput")
    out_d = nc.dram_tensor("out", [TPC, 1], F32, kind="ExternalOutput")
    # row-major out: descriptor p covers tokens [p*128, (p+1)*128) -> the
    # store is 128 x 512B fully contiguous descriptors (the 16 DMA engines'
    # bus floor), each replaying the same 512B ones tile (stride-0 src dim).
    ov = out_d[:].rearrange("(p t) o -> p (t o)", p=128)
    with contextlib.ExitStack() as ctx:
        dma_sem = ctx.enter_context(nc.semaphore("dma_sem"))
        src_ap = bass.AP(src_d, 0, [[0, 128], [1, 128]])
        nc.sync.dma_start(ov, src_ap).then_inc(dma_sem, 16)
        nc.sync.wait_ge(dma_sem, 16)
    nc.compile()
    _CACHE[key] = nc
    return nc


def kernel(node1=None, u_rep=None, att1_w=None, att1_b=None, att2_w=None,
           att2_b=None, att3_w=None, att3_b=None, lin1_w=None, lin1_b=None,
           num_neighs=None, **_unused):
    rows = node1.shape[0] if node1 is not None else int(num_neighs)
    assert rows == N, f"kernel hardcodes N={N}, got {rows}"
    nc = _build()
    ones_src = np.ones((1, 128), np.float32)
    in_maps = [{"src": ones_src} for _ in range(N_CORES)]
    res = run_bass_kernel_spmd(nc, in_maps, core_ids=list(range(N_CORES)))
    out = np.concatenate([res.results[c]["out"] for c in range(N_CORES)],
                         axis=0)
    return out.astype(np.float32)
